# revision 8
# baseline (speedup 1.0000x reference)
"""Censored-loss kernel for Trainium2, data-parallel over 8 NeuronCores.

Math (per reference):
    per_t = targets.sum(-1)                      # [B, T]
    mask  = prefix mask: mask[t] = 1 iff any per_t[t'] > 0 for t' >= t
    censor_p = 1 - outputs.sum(-1)
    loss  = sum(mask * (targets[:,:,0]*ln(censor_p+eps)
                        + sum_v targets[:,:,1+v]*ln(outputs[:,:,v]+eps)))
    count = sum(mask)
    result = -loss / max(count, 1)   (0 if count == 0)

Key simplifications (targets >= 0 by construction):
  * Positions with mask==0 have targets==0 exactly, so they contribute 0 to
    the loss numerator -> no mask needed for the loss sum.
  * count = #positions whose targets are nonzero; we count t0 > 0.

Inputs staged to fp16 on host (halves HBM traffic); targets reordered to
[t0-block | t1..t4 blocks] per row so all on-chip accesses are contiguous.

The kernel is software-pipelined 4 deep so the censor dependency chain
(DVE s2 -> GpSimd s -> ACT Ln -> DVE prod) never serializes a period:
each loop iteration issues  load(i+3) | censor(i+2) | logs(i+1) | main(i).

Engine split per 128-row tile (16 tiles per core), all at or below the
~2.95us/tile DMA floor:
  DVE   (~2.9us): s2 = pairwise censor add (fp16 TT 2x), prod = tg*logt
                  (fp16 TT 2x), count = tensor_scalar is_gt + f32 accum
  GpSimd(~2.1us): s = s2a+s2b, fold01 = prod_c0+prod_c1 (both TT add)
  ACT   (~2.7us): logt[T:] = Ln(o+eps), logt[:T] = Ln(1-s)
  PE    (~2.9us): 4 accumulating ones-matmuls (c2, c3, c4, fold01) into
                  2 alternating [1, 512] f32 PSUM banks
Host: f64 reduction of the [1,1024] loss partials and [P,16] count
partials, then -loss/max(count,1).
"""

import sys

if "/opt/trn_rl_repo" not in sys.path:
    sys.path.insert(0, "/opt/trn_rl_repo")

import numpy as np

import concourse.bacc as bacc
import concourse.mybir as mybir
import concourse.tile as tile
from concourse.bass_utils import run_bass_kernel_spmd

N_CORES = 8
B, T, V = 16384, 512, 5
ROWS = B // N_CORES           # rows per core
P = 128                       # SBUF partitions
NTILES = ROWS // P            # tiles per core
OW = T * (V - 1)              # outputs row width (flattened)
TW = T * V                    # targets row width (flattened)
EPS = 1e-8
F32 = mybir.dt.float32
F16 = mybir.dt.float16
BF16 = mybir.dt.bfloat16
NPF16 = np.float16
ACT = mybir.ActivationFunctionType
ALU = mybir.AluOpType


def build_nc(rows=ROWS):
    ntiles = rows // P
    nc = bacc.Bacc("TRN2", debug=False, num_devices=N_CORES)
    o_d = nc.dram_tensor("outputs", [rows, OW], F16, kind="ExternalInput")
    t_d = nc.dram_tensor("targets", [rows, TW], F16, kind="ExternalInput")
    loss_d = nc.dram_tensor("loss_acc", [1, 2 * T], F32, kind="ExternalOutput")
    cnt_d = nc.dram_tensor("cnt_acc", [P, ntiles], F32, kind="ExternalOutput")

    o_tiled = o_d.ap().rearrange("(n p) m -> n p m", p=P)
    t_tiled = t_d.ap().rearrange("(n p) m -> n p m", p=P)

    with tile.TileContext(nc) as tc:
        with (
            tc.tile_pool(name="inp", bufs=6) as inp,
            tc.tile_pool(name="mid", bufs=4) as mid,
            tc.tile_pool(name="big", bufs=3) as big,
            tc.tile_pool(name="sml", bufs=3) as sml,
            tc.tile_pool(name="acc", bufs=1) as accp,
            tc.tile_pool(name="ps", bufs=1, space="PSUM") as psp,
        ):
            cnt_acc = accp.tile([P, ntiles], F32)
            eps_b = accp.tile([P, 1], F32)
            nc.vector.memset(eps_b[:], EPS)
            ones = accp.tile([P, 1], BF16)
            nc.vector.memset(ones[:], 1.0)
            # two alternating loss accumulators (separate PSUM banks so
            # consecutive accumulating matmuls can pipeline)
            loss_ps0 = psp.tile([1, T], F32, tag="lps0")
            loss_ps1 = psp.tile([1, T], F32, tag="lps1")
            loss_ps = [loss_ps0, loss_ps1]
            nmm = 0  # loss matmul counter across the whole kernel
            n_loss_mm = 4 * ntiles

            o_t, tg_t, s_t, logt_t = {}, {}, {}, {}

            def stage_load(i):
                o = inp.tile([P, OW], F16, tag="o")
                nc.sync.dma_start(o[:], o_tiled[i])
                tg = inp.tile([P, TW], F16, tag="tg")
                nc.sync.dma_start(tg[:], t_tiled[i])
                o_t[i], tg_t[i] = o, tg

            def stage_censor(i):
                # s2[p, t, 0:2] = (o0+o2, o1+o3): consecutive-pair adds in
                # fp16 hit the DVE 2x packed mode
                o = o_t[i]
                s2 = mid.tile([P, T * 2], F16, tag="s2")
                s2v = s2[:].rearrange("p (t v) -> p t v", v=2)
                o3 = o[:].rearrange("p (t v) -> p t v", v=V - 1)
                nc.vector.tensor_tensor(
                    s2v, o3[:, :, 0:2], o3[:, :, 2:4], op=ALU.add
                )
                # full censor sum on the otherwise-idle GpSimd engine
                s = mid.tile([P, T], F16, tag="s")
                nc.gpsimd.tensor_tensor(
                    s[:], s2v[:, :, 0], s2v[:, :, 1], op=ALU.add
                )
                s_t[i] = s

            def stage_logs(i):
                # log tile, same [t0|tv] layout as the reordered targets
                o, s = o_t[i], s_t.pop(i)
                logt = big.tile([P, TW], F16, tag="logt")
                nc.scalar.activation(
                    logt[:][:, T:TW], o[:], ACT.Ln, bias=eps_b[:]
                )
                # f32(1 + 1e-8) == 1.0 exactly, so bias=1.0 == 1+eps
                nc.scalar.activation(
                    logt[:][:, 0:T], s[:], ACT.Ln, bias=1.0, scale=-1.0
                )
                logt_t[i] = logt

            def stage_main(i):
                nonlocal nmm
                o, tg, logt = o_t.pop(i), tg_t.pop(i), logt_t.pop(i)

                # loss product (DVE fp16 2x): prod = targets * logt
                prod = big.tile([P, TW], BF16, tag="prod")
                nc.vector.tensor_tensor(prod[:], tg[:], logt[:], op=ALU.mult)

                # count: is_gt with free-axis f32 accumulation (off the
                # critical path, so emitted after prod)
                sgn = sml.tile([P, T], BF16, tag="sgn")
                nc.vector.tensor_scalar(
                    out=sgn[:], in0=tg[:][:, 0:T],
                    scalar1=0.0, scalar2=0.0, op0=ALU.is_gt, op1=ALU.add,
                    accum_out=cnt_acc[:, i : i + 1],
                )

                # fold chunks 0+1 on GpSimd so PE does 4 matmuls, not 5
                fold = sml.tile([P, T], BF16, tag="fold")
                nc.gpsimd.tensor_tensor(
                    fold[:], prod[:][:, 0:T], prod[:][:, T : 2 * T],
                    op=ALU.add,
                )

                # PE: accumulate partition+chunk sums into PSUM [1, T] accs;
                # fold's matmul goes last so PE starts on c2 right away
                rhss = [
                    prod[:][:, c * T : (c + 1) * T] for c in range(2, V)
                ] + [fold[:]]
                for rhs in rhss:
                    nc.tensor.matmul(
                        loss_ps[nmm % 2][:],
                        ones[:],
                        rhs,
                        start=(nmm < 2),
                        stop=(nmm >= n_loss_mm - 2),
                    )
                    nmm += 1

            # software pipeline, 4 stages deep
            for i in range(ntiles + 3):
                if i < ntiles:
                    stage_load(i)
                if 1 <= i and i - 1 < ntiles:
                    stage_censor(i - 1)
                if 2 <= i and i - 2 < ntiles:
                    stage_logs(i - 2)
                if 3 <= i:
                    stage_main(i - 3)

            loss_sb = accp.tile([1, 2 * T], F32)
            nc.scalar.copy(loss_sb[:, 0:T], loss_ps[0][:])
            nc.scalar.copy(loss_sb[:, T : 2 * T], loss_ps[1][:])
            nc.sync.dma_start(loss_d.ap(), loss_sb[:])
            nc.sync.dma_start(cnt_d.ap(), cnt_acc[:])
    nc.compile()
    return nc


_NC_CACHE = {}


def _get_nc(rows=ROWS):
    if rows not in _NC_CACHE:
        _NC_CACHE[rows] = build_nc(rows)
    return _NC_CACHE[rows]


def pack_inputs(outputs, targets):
    """fp16 staging + per-row [t0-block | tv-block] reorder of targets."""
    o = np.asarray(outputs).reshape(N_CORES, ROWS, OW).astype(NPF16)
    t3 = np.asarray(targets).reshape(N_CORES, ROWS, T, V).astype(NPF16)
    tg = np.concatenate(
        [t3[:, :, :, 0], t3[:, :, :, 1:].reshape(N_CORES, ROWS, OW)], axis=2
    )
    return o, tg


def run_spmd(outputs, targets, trace=False, **kwargs):
    o, tg = pack_inputs(outputs, targets)
    in_maps = [{"outputs": o[k], "targets": tg[k]} for k in range(N_CORES)]
    nc = _get_nc()
    res = run_bass_kernel_spmd(
        nc, in_maps, core_ids=list(range(N_CORES)), trace=trace, **kwargs
    )
    loss = sum(r["loss_acc"].astype(np.float64).sum() for r in res.results)
    cnt = sum(r["cnt_acc"].astype(np.float64).sum() for r in res.results)
    return loss, cnt, res


def kernel(outputs, targets):
    loss, cnt, _ = run_spmd(outputs, targets)
    if cnt > 0:
        return np.float32(-loss / max(cnt, 1.0))
    return np.float32(0.0)


# revision 12
# speedup vs baseline: 1.0483x; 1.0483x over previous
"""Censored-loss kernel for Trainium2, data-parallel over 8 NeuronCores.

Math (per reference):
    per_t = targets.sum(-1)                      # [B, T]
    mask  = prefix mask: mask[t] = 1 iff any per_t[t'] > 0 for t' >= t
    censor_p = 1 - outputs.sum(-1)
    loss  = sum(mask * (targets[:,:,0]*ln(censor_p+eps)
                        + sum_v targets[:,:,1+v]*ln(outputs[:,:,v]+eps)))
    count = sum(mask)
    result = -loss / max(count, 1)   (0 if count == 0)

Key simplifications (targets >= 0 by construction):
  * Positions with mask==0 have targets==0 exactly, so they contribute 0 to
    the loss numerator -> no mask needed for the loss sum.
  * count = #positions whose targets are nonzero; we count t0 > 0.

Host staging: fp16 (halves HBM traffic); targets reordered per row to
[t0 | t1 | t2 | t3 | t4] blocks and outputs to v-major [o1 | o2 | o3 | o4]
blocks, so every on-chip access -- including both censor-sum adds -- is
contiguous and hits the DVE 2x packed mode.

Software-pipelined 4 deep (load | censor | logs | main(+lagged fold-MMs))
so the censor chain never serializes a period. GpSimd carries only
strictly-downstream work (the two prod folds); anything upstream on its
in-order queue would back-couple the pipeline (measured: +24us).

Engine budget per 128-row tile (16/core), DMA floor ~2.95us/tile:
  DVE   (~2.9us): s2 = o_lo+o_hi (2x), s = s2_lo+s2_hi (2x),
                  prod = tg*logt (2x), sgn = is_gt(t0,0) (4x)
  ACT   (~2.7us): logt[T:] = Ln(o+eps), logt[:T] = Ln(1-s)
  GpSimd(~2.1us): fold01 = pc0+pc1, fold23 = pc2+pc3 (fp16 TT adds)
  PE    (~2.9us): 4 matmuls: count(sgn), c4, fold01, fold23 (folds lag
                  one stage) into 2 loss banks + 1 count bank of PSUM
Host: f64 reduction of [1, 3T] partials, then -loss/max(count,1).
"""

import sys

if "/opt/trn_rl_repo" not in sys.path:
    sys.path.insert(0, "/opt/trn_rl_repo")

import numpy as np

import concourse.bacc as bacc
import concourse.mybir as mybir
import concourse.tile as tile
from concourse.bass_utils import run_bass_kernel_spmd

N_CORES = 8
B, T, V = 16384, 512, 5
ROWS = B // N_CORES           # rows per core
P = 128                       # SBUF partitions
NTILES = ROWS // P            # tiles per core
OW = T * (V - 1)              # outputs row width (flattened)
TW = T * V                    # targets row width (flattened)
EPS = 1e-8
F32 = mybir.dt.float32
F16 = mybir.dt.float16
BF16 = mybir.dt.bfloat16
NPF16 = np.float16
ACT = mybir.ActivationFunctionType
ALU = mybir.AluOpType


def build_nc(rows=ROWS):
    ntiles = rows // P
    nc = bacc.Bacc("TRN2", debug=False, num_devices=N_CORES)
    o_d = nc.dram_tensor("outputs", [rows, OW], F16, kind="ExternalInput")
    t_d = nc.dram_tensor("targets", [rows, TW], F16, kind="ExternalInput")
    loss_d = nc.dram_tensor("loss_acc", [1, 3 * T], F32, kind="ExternalOutput")

    o_tiled = o_d.ap().rearrange("(n p) m -> n p m", p=P)
    t_tiled = t_d.ap().rearrange("(n p) m -> n p m", p=P)

    with tile.TileContext(nc) as tc:
        with (
            tc.tile_pool(name="inp", bufs=6) as inp,
            tc.tile_pool(name="mid", bufs=4) as mid,
            tc.tile_pool(name="big", bufs=3) as big,
            tc.tile_pool(name="sml", bufs=4) as sml,
            tc.tile_pool(name="acc", bufs=1) as accp,
            tc.tile_pool(name="ps", bufs=1, space="PSUM") as psp,
        ):
            eps_b = accp.tile([P, 1], F32)
            nc.vector.memset(eps_b[:], EPS)
            ones = accp.tile([P, 1], BF16)
            nc.vector.memset(ones[:], 1.0)
            # two alternating loss accumulators (separate PSUM banks so
            # consecutive accumulating matmuls can pipeline) + count bank
            loss_ps0 = psp.tile([1, T], F32, tag="lps0")
            loss_ps1 = psp.tile([1, T], F32, tag="lps1")
            loss_ps = [loss_ps0, loss_ps1]
            cnt_ps = psp.tile([1, T], F32, tag="cps")
            nmm = 0  # loss matmul counter across the whole kernel
            n_loss_mm = 3 * ntiles

            o_t, tg_t, s_t, sgn_t, logt_t, fold_t = {}, {}, {}, {}, {}, {}

            def stage_load(i):
                o = inp.tile([P, OW], F16, tag="o")
                nc.sync.dma_start(o[:], o_tiled[i])
                tg = inp.tile([P, TW], F16, tag="tg")
                nc.sync.dma_start(tg[:], t_tiled[i])
                o_t[i], tg_t[i] = o, tg

            def stage_censor(i):
                # v-major halves-adds: both contiguous, DVE 2x packed mode
                o, tg = o_t[i], tg_t[i]
                s2 = mid.tile([P, T * 2], F16, tag="s2")
                nc.vector.tensor_tensor(
                    s2[:], o[:][:, 0 : 2 * T], o[:][:, 2 * T : 4 * T],
                    op=ALU.add,
                )
                s = mid.tile([P, T], F16, tag="s")
                nc.vector.tensor_tensor(
                    s[:], s2[:][:, 0:T], s2[:][:, T : 2 * T], op=ALU.add
                )
                s_t[i] = s
                # count mask: is_gt at DVE 4x; summed later by a PE matmul
                sgn = sml.tile([P, T], BF16, tag="sgn")
                nc.vector.tensor_scalar(
                    out=sgn[:], in0=tg[:][:, 0:T],
                    scalar1=0.0, scalar2=None, op0=ALU.is_gt,
                )
                sgn_t[i] = sgn

            def stage_logs(i):
                # log tile, same [t0|t1..t4] block layout as targets
                o, s = o_t[i], s_t.pop(i)
                logt = big.tile([P, TW], F16, tag="logt")
                nc.scalar.activation(
                    logt[:][:, T:TW], o[:], ACT.Ln, bias=eps_b[:]
                )
                # f32(1 + 1e-8) == 1.0 exactly, so bias=1.0 == 1+eps
                nc.scalar.activation(
                    logt[:][:, 0:T], s[:], ACT.Ln, bias=1.0, scale=-1.0
                )
                logt_t[i] = logt

            def stage_main(i):
                nonlocal nmm
                o, tg, logt = o_t.pop(i), tg_t.pop(i), logt_t.pop(i)
                sgn = sgn_t.pop(i)

                # loss product (DVE fp16 2x): prod = targets * logt
                prod = big.tile([P, TW], F16, tag="prod")
                nc.vector.tensor_tensor(prod[:], tg[:], logt[:], op=ALU.mult)

                # two folds on GpSimd (strictly downstream of prod)
                f01 = sml.tile([P, T], F16, tag="f01")
                nc.gpsimd.tensor_tensor(
                    f01[:], prod[:][:, 0:T], prod[:][:, T : 2 * T],
                    op=ALU.add,
                )
                f23 = sml.tile([P, T], F16, tag="f23")
                nc.gpsimd.tensor_tensor(
                    f23[:], prod[:][:, 2 * T : 3 * T],
                    prod[:][:, 3 * T : 4 * T], op=ALU.add,
                )
                fold_t[i] = (f01, f23)

                # PE now: count matmul + last prod chunk
                nc.tensor.matmul(
                    cnt_ps[:], ones[:], sgn[:],
                    start=(i == 0), stop=(i == ntiles - 1),
                )
                nc.tensor.matmul(
                    loss_ps[nmm % 2][:], ones[:],
                    prod[:][:, 4 * T : 5 * T],
                    start=(nmm < 2), stop=(nmm >= n_loss_mm - 2),
                )
                nmm += 1

            def stage_foldmm(i):
                nonlocal nmm
                f01, f23 = fold_t.pop(i)
                for rhs in (f01, f23):
                    nc.tensor.matmul(
                        loss_ps[nmm % 2][:], ones[:], rhs[:],
                        start=(nmm < 2), stop=(nmm >= n_loss_mm - 2),
                    )
                    nmm += 1

            # software pipeline, 5 stages deep
            for i in range(ntiles + 4):
                if i < ntiles:
                    stage_load(i)
                if 1 <= i and i - 1 < ntiles:
                    stage_censor(i - 1)
                if 2 <= i and i - 2 < ntiles:
                    stage_logs(i - 2)
                if 3 <= i and i - 3 < ntiles:
                    stage_main(i - 3)
                if 4 <= i:
                    stage_foldmm(i - 4)

            loss_sb = accp.tile([1, 3 * T], F32)
            nc.scalar.copy(loss_sb[:, 0:T], loss_ps[0][:])
            nc.scalar.copy(loss_sb[:, T : 2 * T], loss_ps[1][:])
            nc.scalar.copy(loss_sb[:, 2 * T : 3 * T], cnt_ps[:])
            nc.sync.dma_start(loss_d.ap(), loss_sb[:])
    nc.compile()
    return nc


_NC_CACHE = {}


def _get_nc(rows=ROWS):
    if rows not in _NC_CACHE:
        _NC_CACHE[rows] = build_nc(rows)
    return _NC_CACHE[rows]


def pack_inputs(outputs, targets):
    """fp16 staging; per-row v-major block reorder of both tensors."""
    o4 = np.asarray(outputs).reshape(N_CORES, ROWS, T, V - 1).astype(NPF16)
    o = o4.transpose(0, 1, 3, 2).reshape(N_CORES, ROWS, OW)
    t5 = np.asarray(targets).reshape(N_CORES, ROWS, T, V).astype(NPF16)
    tg = t5.transpose(0, 1, 3, 2).reshape(N_CORES, ROWS, TW)
    return np.ascontiguousarray(o), np.ascontiguousarray(tg)


def run_spmd(outputs, targets, trace=False, **kwargs):
    o, tg = pack_inputs(outputs, targets)
    in_maps = [{"outputs": o[k], "targets": tg[k]} for k in range(N_CORES)]
    nc = _get_nc()
    res = run_bass_kernel_spmd(
        nc, in_maps, core_ids=list(range(N_CORES)), trace=trace, **kwargs
    )
    loss = sum(
        r["loss_acc"][:, : 2 * T].astype(np.float64).sum()
        for r in res.results
    )
    cnt = sum(
        r["loss_acc"][:, 2 * T :].astype(np.float64).sum()
        for r in res.results
    )
    return loss, cnt, res


def kernel(outputs, targets):
    loss, cnt, _ = run_spmd(outputs, targets)
    if cnt > 0:
        return np.float32(-loss / max(cnt, 1.0))
    return np.float32(0.0)


# revision 13
# speedup vs baseline: 1.1209x; 1.0693x over previous
"""Censored-loss kernel for Trainium2, data-parallel over 8 NeuronCores.

Math (per reference):
    per_t = targets.sum(-1)                      # [B, T]
    mask  = prefix mask: mask[t] = 1 iff any per_t[t'] > 0 for t' >= t
    censor_p = 1 - outputs.sum(-1)
    loss  = sum(mask * (targets[:,:,0]*ln(censor_p+eps)
                        + sum_v targets[:,:,1+v]*ln(outputs[:,:,v]+eps)))
    count = sum(mask)
    result = -loss / max(count, 1)   (0 if count == 0)

Key simplifications (targets >= 0 by construction):
  * Positions with mask==0 have targets==0 exactly, so they contribute 0 to
    the loss numerator -> no mask needed for the loss sum.
  * count = #positions whose targets are nonzero; we count t0 > 0 via
    tensor_scalar is_gt with f32 accum_out (no PE matmul, no ACT Sign).

Host staging: fp16 (halves HBM traffic); targets reordered per row to
[t0 | t1..t4] blocks; outputs kept v-interleaved for the 2x packed
pair-add.

Software-pipelined 4 deep (load | censor | logs | main) so the censor
chain (DVE s2 -> GpSimd s -> ACT Ln -> DVE prod) spans stages instead of
serializing a period. GpSimd carries only upstream work (s); mixing
upstream+downstream ops on its in-order queue back-couples the pipeline
(measured +24us). All op shapes chosen are ones measured at full speed
under steady-state load (some 512-wide DVE variants degrade 3x there).

Engine budget per 128-row tile (16/core), DMA floor ~2.95us/tile:
  DVE   (~2.9us): s2 pair-add (2x), prod = tg*logt (2x), count TS+accum
  GpSimd(~1.0us): s = s2a+s2b
  ACT   (~2.7us): logt[T:] = Ln(o+eps), logt[:T] = Ln(1-s)
  PE    (~2.5-3.5us): 5 ones-matmuls (c0..c4) into 4 rotating PSUM banks
Host: f64 reduction of the [1, 4T] loss partials and [P, 16] count
partials, then -loss/max(count,1).
"""

import sys

if "/opt/trn_rl_repo" not in sys.path:
    sys.path.insert(0, "/opt/trn_rl_repo")

import numpy as np

import concourse.bacc as bacc
import concourse.mybir as mybir
import concourse.tile as tile
from concourse.bass_utils import run_bass_kernel_spmd

N_CORES = 8
B, T, V = 16384, 512, 5
ROWS = B // N_CORES           # rows per core
P = 128                       # SBUF partitions
NTILES = ROWS // P            # tiles per core
OW = T * (V - 1)              # outputs row width (flattened)
TW = T * V                    # targets row width (flattened)
EPS = 1e-8
F32 = mybir.dt.float32
F16 = mybir.dt.float16
BF16 = mybir.dt.bfloat16
NPF16 = np.float16
ACT = mybir.ActivationFunctionType
ALU = mybir.AluOpType


def build_nc(rows=ROWS):
    ntiles = rows // P
    nc = bacc.Bacc("TRN2", debug=False, num_devices=N_CORES)
    o_d = nc.dram_tensor("outputs", [rows, OW], F16, kind="ExternalInput")
    t_d = nc.dram_tensor("targets", [rows, TW], F16, kind="ExternalInput")
    loss_d = nc.dram_tensor("loss_acc", [1, 4 * T], F32, kind="ExternalOutput")
    cnt_d = nc.dram_tensor("cnt_acc", [P, ntiles], F32, kind="ExternalOutput")

    o_tiled = o_d.ap().rearrange("(n p) m -> n p m", p=P)
    t_tiled = t_d.ap().rearrange("(n p) m -> n p m", p=P)

    with tile.TileContext(nc) as tc:
        with (
            tc.tile_pool(name="inp", bufs=6) as inp,
            tc.tile_pool(name="mid", bufs=4) as mid,
            tc.tile_pool(name="big", bufs=3) as big,
            tc.tile_pool(name="acc", bufs=1) as accp,
            tc.tile_pool(name="ps", bufs=1, space="PSUM") as psp,
        ):
            cnt_acc = accp.tile([P, ntiles], F32)
            eps_b = accp.tile([P, 1], F32)
            nc.vector.memset(eps_b[:], EPS)
            ones = accp.tile([P, 1], BF16)
            nc.vector.memset(ones[:], 1.0)
            # four rotating loss accumulators (separate PSUM banks so
            # consecutive accumulating matmuls can pipeline)
            loss_ps0 = psp.tile([1, T], F32, tag="lps0")
            loss_ps1 = psp.tile([1, T], F32, tag="lps1")
            loss_ps2 = psp.tile([1, T], F32, tag="lps2")
            loss_ps3 = psp.tile([1, T], F32, tag="lps3")
            loss_ps = [loss_ps0, loss_ps1, loss_ps2, loss_ps3]
            NB = len(loss_ps)
            nmm = 0  # loss matmul counter across the whole kernel
            n_loss_mm = 5 * ntiles

            o_t, tg_t, s_t, logt_t = {}, {}, {}, {}

            def stage_load(i):
                o = inp.tile([P, OW], F16, tag="o")
                nc.sync.dma_start(o[:], o_tiled[i])
                tg = inp.tile([P, TW], F16, tag="tg")
                nc.sync.dma_start(tg[:], t_tiled[i])
                o_t[i], tg_t[i] = o, tg

            def stage_censor(i):
                # s2[p, t, 0:2] = (o0+o2, o1+o3): consecutive-pair adds in
                # fp16 hit the DVE 2x packed mode
                o = o_t[i]
                s2 = mid.tile([P, T * 2], F16, tag="s2")
                s2v = s2[:].rearrange("p (t v) -> p t v", v=2)
                o3 = o[:].rearrange("p (t v) -> p t v", v=V - 1)
                nc.vector.tensor_tensor(
                    s2v, o3[:, :, 0:2], o3[:, :, 2:4], op=ALU.add
                )
                # full censor sum on the otherwise-idle GpSimd engine
                s = mid.tile([P, T], F16, tag="s")
                nc.gpsimd.tensor_tensor(
                    s[:], s2v[:, :, 0], s2v[:, :, 1], op=ALU.add
                )
                s_t[i] = s

            def stage_logs(i):
                # log tile, same [t0 | tv] layout as the reordered targets
                o, s = o_t[i], s_t.pop(i)
                logt = big.tile([P, TW], F16, tag="logt")
                nc.scalar.activation(
                    logt[:][:, T:TW], o[:], ACT.Ln, bias=eps_b[:]
                )
                # f32(1 + 1e-8) == 1.0 exactly, so bias=1.0 == 1+eps
                nc.scalar.activation(
                    logt[:][:, 0:T], s[:], ACT.Ln, bias=1.0, scale=-1.0
                )
                logt_t[i] = logt

            def stage_main(i):
                nonlocal nmm
                o, tg, logt = o_t.pop(i), tg_t.pop(i), logt_t.pop(i)

                # loss product (DVE fp16 2x): prod = targets * logt
                prod = big.tile([P, TW], BF16, tag="prod")
                nc.vector.tensor_tensor(prod[:], tg[:], logt[:], op=ALU.mult)

                # count: is_gt with f32 accumulation (1x, but proven immune
                # to the under-load DVE slowdown)
                sgn = mid.tile([P, T], BF16, tag="sgn")
                nc.vector.tensor_scalar(
                    out=sgn[:], in0=tg[:][:, 0:T],
                    scalar1=0.0, scalar2=0.0, op0=ALU.is_gt, op1=ALU.add,
                    accum_out=cnt_acc[:, i : i + 1],
                )

                # PE: 5 back-to-back chunk matmuls (burst keeps PE pstate
                # high) into rotating PSUM banks
                for c in range(V):
                    nc.tensor.matmul(
                        loss_ps[nmm % NB][:], ones[:],
                        prod[:][:, c * T : (c + 1) * T],
                        start=(nmm < NB), stop=(nmm >= n_loss_mm - NB),
                    )
                    nmm += 1

            # software pipeline, 4 stages deep
            for i in range(ntiles + 3):
                if i < ntiles:
                    stage_load(i)
                if 1 <= i and i - 1 < ntiles:
                    stage_censor(i - 1)
                if 2 <= i and i - 2 < ntiles:
                    stage_logs(i - 2)
                if 3 <= i:
                    stage_main(i - 3)

            loss_sb = accp.tile([1, 4 * T], F32)
            for b in range(NB):
                nc.scalar.copy(
                    loss_sb[:, b * T : (b + 1) * T], loss_ps[b][:]
                )
            nc.sync.dma_start(loss_d.ap(), loss_sb[:])
            nc.sync.dma_start(cnt_d.ap(), cnt_acc[:])
    nc.compile()
    return nc


_NC_CACHE = {}


def _get_nc(rows=ROWS):
    if rows not in _NC_CACHE:
        _NC_CACHE[rows] = build_nc(rows)
    return _NC_CACHE[rows]


def pack_inputs(outputs, targets):
    """fp16 staging + per-row [t0-block | tv-block] reorder of targets."""
    o = np.asarray(outputs).reshape(N_CORES, ROWS, OW).astype(NPF16)
    t3 = np.asarray(targets).reshape(N_CORES, ROWS, T, V).astype(NPF16)
    tg = np.concatenate(
        [t3[:, :, :, 0], t3[:, :, :, 1:].reshape(N_CORES, ROWS, OW)], axis=2
    )
    return o, tg


def run_spmd(outputs, targets, trace=False, **kwargs):
    o, tg = pack_inputs(outputs, targets)
    in_maps = [{"outputs": o[k], "targets": tg[k]} for k in range(N_CORES)]
    nc = _get_nc()
    res = run_bass_kernel_spmd(
        nc, in_maps, core_ids=list(range(N_CORES)), trace=trace, **kwargs
    )
    loss = sum(r["loss_acc"].astype(np.float64).sum() for r in res.results)
    cnt = sum(r["cnt_acc"].astype(np.float64).sum() for r in res.results)
    return loss, cnt, res


def kernel(outputs, targets):
    loss, cnt, _ = run_spmd(outputs, targets)
    if cnt > 0:
        return np.float32(-loss / max(cnt, 1.0))
    return np.float32(0.0)
